# revision 7
# baseline (speedup 1.0000x reference)
"""Trainium2 Bass kernel for batched two-layer-MLP attention.

Reference semantics (per batch b):
    x  = sequence[:, b, :]                        # [S, D]
    K  = tanh(tanh(x @ Kw1.T) @ Kw2.T)
    Q  = tanh(tanh(x @ Qw1.T) @ Qw2.T)
    W  = softmax(K @ Q.T / sqrt(D), axis=-1)      # [S, S]
    out[:, b, :] = W @ x
Sharding: data-parallel over batch (B=8 -> 8 NeuronCores), weights replicated.

This version runs the matmuls in fp8(e4m3) with DoubleRow perf mode: the PE
packs 2 fp8 weights per cell, contracting 256 per instruction at ~0.5
cycles/row -- ~1.4-1.8x the bf16 matmul rate. Operand layouts keep k-pairs
adjacent so each DR matmul slices [128, 2, N] 3D APs out of the same tiles
the bf16 version used. Weights are pre-scaled x32 on the host so their
~N(0, 1/32) entries land in e4m3's normal range; the 1/32 is folded into the
layer's tanh activation scale. Accuracy (vs the 2e-2 gate): e4m3 quantization
of both operands adds ~2-3% per-element error per matmul, but tanh/softmax
compression keeps the end-to-end error in budget (verified numerically).

Layout strategy per core (as the bf16 baseline):
  - xt = x.T [D, S] fp8 host-pretransposed; weights pre-packed [p, j, k, c]
  - MLP outputs stay transposed: Kt, Qt [D, S] fp8
  - scores SC[s,t]: lhsT = Kt tile, rhs = Qt chunk; exp's accum_out -> rowsums
  - exp(SC) row-block bf16 -> ONE xbar DMA-transpose (2-byte only) -> DVE
    converts bf16->fp8 into lhsT layout for attended = Wt.T @ x, rhs = xn fp8
  - 1/rowsum folded into the PSUM->SBUF copy of the output

Scheduling: HAM warmup matmuls fill the initial DMA wait; first-layer inputs
k/j-split across both HWDGE rings; phase B scores PSUM pool opened early;
phase B software-pipelined one row-block ahead.
"""

import numpy as np
import ml_dtypes

import concourse.bacc as bacc
import concourse.tile as tile
from concourse import mybir
from concourse.bass_utils import run_bass_kernel_spmd

P = 128          # partitions
S = 2048         # sequence length
D = 1024         # model dim
B = 8            # batch (one per core)
ST = S // P      # 16 s-tiles
DT = D // P      # 8 d-tiles
NF = 512         # psum free width (one bank of fp32)
SN = S // NF     # 4 score free-chunks
DN = D // NF     # 2 output free-chunks
BF = mybir.dt.bfloat16
F32 = mybir.dt.float32
E4 = mybir.dt.float8e4
SCALE = 1.0 / np.sqrt(np.float32(D))
WS = 32.0        # host-side weight pre-scale before fp8 cast

# per-stage precision switches (fp8 DoubleRow vs bf16)
MLP_FP8 = False
SC_FP8 = True
AT_FP8 = True
WC = 1.0         # attended-matmul W centering: quantize (exp(sc) - WC) in fp8;
                 # the rank-1 term WC * outer(1/rowsum, colsum(x)) is added
                 # back on the host (softmax logits are near-uniform, so
                 # centering shrinks fp8 quantization error ~2.5x)

DR = mybir.MatmulPerfMode.DoubleRow
AX = mybir.AxisListType.X
AF = mybir.ActivationFunctionType

MLP_DT = E4 if MLP_FP8 else BF
SC_DT = E4 if SC_FP8 else BF
AT_DT = E4 if AT_FP8 else BF


def dr_matmuls(nc, ps, lhs_sl, rhs_sl, nk, fp8):
    """Accumulate nk 128-deep k-slices into ps; paired DoubleRow when fp8."""
    if fp8:
        for k2 in range(nk // 2):
            nc.tensor.matmul(ps, lhs_sl(2 * k2, 2), rhs_sl(2 * k2, 2),
                             start=(k2 == 0), stop=(k2 == nk // 2 - 1),
                             perf_mode=DR)
    else:
        for k in range(nk):
            nc.tensor.matmul(ps, lhs_sl(k, 1), rhs_sl(k, 1),
                             start=(k == 0), stop=(k == nk - 1))


def build_nc():
    nc = bacc.Bacc("TRN2", target_bir_lowering=False)

    xt_d = nc.dram_tensor("xt", [D, S], MLP_DT, kind="ExternalInput")
    xn_d = nc.dram_tensor("xn", [S, D], AT_DT, kind="ExternalInput")
    # head tensor: x.T's n=0 chunk pre-packed [p, kh, k%, s] so each k-half
    # loads with one fully-contiguous DMA (first matmul fires earliest)
    xh_d = nc.dram_tensor("xh", [P, 2, DT // 2, NF], MLP_DT, kind="ExternalInput")
    # weights pre-arranged on the host to [p, j, k, c] so each j-block loads
    # with one partition-contiguous DMA
    WSHAPE = [P, DT, DT, P]
    wk1_d = nc.dram_tensor("wk1", WSHAPE, MLP_DT, kind="ExternalInput")
    wk2_d = nc.dram_tensor("wk2", WSHAPE, MLP_DT, kind="ExternalInput")
    wq1_d = nc.dram_tensor("wq1", WSHAPE, MLP_DT, kind="ExternalInput")
    wq2_d = nc.dram_tensor("wq2", WSHAPE, MLP_DT, kind="ExternalInput")
    out_d = nc.dram_tensor("out", [S, D], F32, kind="ExternalOutput")
    if AT_FP8:
        # per-row 1/softmax-denominator, exported for the host rank-1 fixup
        rcp_d = nc.dram_tensor("rcpv", [S, 1], F32, kind="ExternalOutput")

    from contextlib import ExitStack

    with tile.TileContext(nc) as tc, ExitStack() as ctx:
        # ---- persistent SBUF arrays (live across both phases) ----
        pers = ctx.enter_context(tc.tile_pool(name="pers", bufs=1))
        xn_sb = pers.tile([P, ST, D], AT_DT)  # x normal: [t-part, t-tile, d]
        kt_sb = pers.tile([P, DT, S], SC_DT)  # K.T: [d-part, d-tile, s]
        # Q.T split per n-chunk so phase B's first scores don't wait on the
        # whole tensor's last tanh
        qt_n = [pers.tile([P, DT, NF], SC_DT, tag=f"qt{n}", name=f"qt{n}")
                for n in range(SN)]

        # scores PSUM pool opened before phase A so it gets banks disjoint
        # from the MLP pool - phase B's first matmul then has no released-pool
        # overlap dependency on phase A's tail
        psc = ctx.enter_context(tc.tile_pool(name="psum_sc", bufs=3, space="PSUM"))

        # ---- phase A: the four MLP layers ----
        with tc.tile_pool(name="phase_a", bufs=1) as pa, \
             tc.tile_pool(name="wpool", bufs=2) as wp, \
             tc.tile_pool(name="psum_mlp", bufs=4, space="PSUM") as pm:
            # x.T split into per-n-chunk tiles so the first psum row's matmuls
            # only wait on the 1MB slice they read, not the whole array;
            # the n=0 chunk is additionally k-halved for an even earlier start
            KH = DT // 2
            xt_f = [pa.tile([P, KH, NF], MLP_DT, tag=f"xtf{h}", name=f"xtf{h}")
                    for h in range(2)]
            xt_n = [pa.tile([P, DT, NF], MLP_DT, tag=f"xt{n}", name=f"xt{n}")
                    for n in range(1, SN)]

            def xt_slice(n, k, w):
                if n == 0:
                    return xt_f[k // KH][:, k % KH:k % KH + w, :]
                return xt_n[n - 1][:, k:k + w, :]

            h1_sb = pa.tile([P, DT, S], MLP_DT)  # hidden activations (K then Q)

            # HAM warmup: throwaway matmuls while the first input DMAs are in
            # flight, so the real matmuls start at 2.4GHz
            warm_sb = pa.tile([P, NF], BF)
            nc.vector.memset(warm_sb, 0.0)
            warm_ps = pm.tile([P, NF], F32, tag="warm", bufs=1)
            NWARM = 13
            for i in range(NWARM):
                nc.tensor.matmul(warm_ps, warm_sb[:, 0:P], warm_sb,
                                 start=(i == 0), stop=(i == NWARM - 1))

            def mlp_layer(src, w_dram, dst, xdma=None, first=False):
                # dst[j, s] = tanh(scale * sum_k w[k, j].T @ src[k, s])
                # one tile + one DMA per j-block so dep granularity is per-j.
                xt_r = xt_d.rearrange("(k p) s -> p k s", p=P)
                if first:
                    w_jf = [wp.tile([P, KH, P], MLP_DT, tag=f"wf{h}", name=f"wf{h}")
                            for h in range(2)]
                    w_j = [wp.tile([P, DT, P], MLP_DT, tag=f"w{j}", name=f"w{j}")
                           for j in range(1, DT)]
                    for h in range(2):
                        nc.sync.dma_start(out=xt_f[h], in_=xh_d[:, h, :, :])
                        nc.scalar.dma_start(
                            out=w_jf[h], in_=w_dram[:, 0, h * KH:(h + 1) * KH, :])
                    for j in range(1, DT):
                        nc.scalar.dma_start(out=w_j[j - 1], in_=w_dram[:, j, :, :])
                    for n in range(1, SN):
                        nc.sync.dma_start(
                            out=xt_n[n - 1], in_=xt_r[:, :, n * NF:(n + 1) * NF])

                    def lhs_sl(j, k, w):
                        return (w_jf[k // KH][:, k % KH:k % KH + w, :] if j == 0
                                else w_j[j - 1][:, k:k + w, :])
                else:
                    w_j = [wp.tile([P, DT, P], MLP_DT, tag=f"w{j}", name=f"w{j}")
                           for j in range(DT)]
                    for j in range(DT):
                        nc.sync.dma_start(out=w_j[j], in_=w_dram[:, j, :, :])
                    if xdma is not None:
                        xdma()

                    def lhs_sl(j, k, w):
                        return w_j[j][:, k:k + w, :]

                def rhs_sl(n, k, w):
                    return (xt_slice(n, k, w) if src is None
                            else src[:, k:k + w, n * NF:(n + 1) * NF])

                loop = ([(j, n) for n in range(SN) for j in range(DT)] if first
                        else [(j, n) for j in range(DT) for n in range(SN)])
                for j, n in loop:
                    ps = pm.tile([P, NF], F32, tag="mlp")
                    dr_matmuls(nc, ps,
                               lambda k, w: lhs_sl(j, k, w),
                               lambda k, w: rhs_sl(n, k, w),
                               DT, MLP_FP8)
                    dslice = (dst[n][:, j, :] if isinstance(dst, list)
                              else dst[:, j, n * NF:(n + 1) * NF])
                    nc.scalar.activation(out=dslice, in_=ps, func=AF.Tanh,
                                         scale=(1.0 / WS) if MLP_FP8 else 1.0)

            def load_xn():
                xn_r = xn_d.rearrange("(t p) d -> p t d", p=P)
                for t in range(0, ST, 4):
                    nc.sync.dma_start(out=xn_sb[:, t:t + 4, :],
                                      in_=xn_r[:, t:t + 4, :])

            mlp_layer(None, wk1_d, h1_sb, first=True)
            mlp_layer(h1_sb, wk2_d, kt_sb)
            mlp_layer(None, wq1_d, h1_sb, xdma=load_xn)
            mlp_layer(h1_sb, wq2_d, qt_n)

        # ---- phase B: scores -> softmax -> transpose -> attended ----
        with tc.tile_pool(name="wexp", bufs=2) as wexp_pool, \
             tc.tile_pool(name="wtT", bufs=2) as wtT_pool, \
             tc.tile_pool(name="wtT8", bufs=2) as wtT8_pool, \
             tc.tile_pool(name="sums", bufs=4) as sums_pool, \
             tc.tile_pool(name="outst", bufs=2) as out_pool, \
             tc.tile_pool(name="psum_at", bufs=3, space="PSUM") as pat:

            def scores_softmax_transpose(i):
                """Row-block i of exp(scores) plus its reciprocal row sums,
                transposed (and fp8-converted) into lhsT layout for the
                attended matmul."""
                wexp = wexp_pool.tile([P, S], BF, tag="wexp")
                sums = sums_pool.tile([P, SN], F32, tag="sums")
                for n in range(SN):
                    ps = psc.tile([P, NF], F32, tag="sc")
                    dr_matmuls(
                        nc, ps,
                        lambda k, w: kt_sb[:, k:k + w, i * P:(i + 1) * P],
                        lambda k, w: qt_n[n][:, k:k + w, :],
                        DT, SC_FP8)
                    # scores are bounded (|sc/32| < ~3): exp without max-shift
                    nc.scalar.activation(
                        out=wexp[:, n * NF:(n + 1) * NF],
                        in_=ps,
                        func=AF.Exp,
                        scale=float(SCALE),
                        accum_out=sums[:, n:n + 1],
                    )
                rcp = sums_pool.tile([P, 1], F32, tag="rcp")
                nc.vector.reduce_sum(rcp, sums, axis=AX)
                nc.vector.reciprocal(rcp, rcp)
                # one xbar transpose of the whole row-block:
                #   wtT[p, t, c] = wexp[c, t*128 + p]
                wtT = wtT_pool.tile([P, ST, P], BF, tag="wtT")
                nc.scalar.dma_start_transpose(out=wtT, in_=wexp)
                if AT_FP8:
                    nc.sync.dma_start(out=rcp_d[i * P:(i + 1) * P, :], in_=rcp)
                    # centered fp8 weights for the DoubleRow attended matmul
                    wtT8 = wtT8_pool.tile([P, ST, P], E4, tag="wtT8")
                    nc.vector.tensor_scalar(wtT8, wtT, float(WC), None,
                                            mybir.AluOpType.subtract)
                    return wtT8, rcp
                return wtT, rcp

            def attended(i, wtT, rcp, last=False):
                outst = out_pool.tile([P, D], F32, tag="outst")
                for n in range(DN):
                    ps = pat.tile([P, NF], F32, tag="at")
                    dr_matmuls(
                        nc, ps,
                        lambda t, w: wtT[:, t:t + w, :],
                        lambda t, w: xn_sb[:, t:t + w, n * NF:(n + 1) * NF],
                        ST, AT_FP8)
                    # fold the softmax normalization into the PSUM->SBUF copy
                    nc.scalar.mul(outst[:, n * NF:(n + 1) * NF], ps, rcp)
                    nc.sync.dma_start(
                        out=out_d[i * P:(i + 1) * P, n * NF:(n + 1) * NF],
                        in_=outst[:, n * NF:(n + 1) * NF],
                    )

            # software-pipelined: attended(i-1) is emitted after scores(i) so
            # the PE never waits on the transpose/convert
            prev = None
            for i in range(ST):
                cur = scores_softmax_transpose(i)
                if prev is not None:
                    attended(i - 1, *prev)
                prev = cur
            attended(ST - 1, *prev, last=True)

    nc.compile()
    return nc


_NC = None


def _get_nc():
    global _NC
    if _NC is None:
        _NC = build_nc()
    return _NC


NP_MLP = ml_dtypes.float8_e4m3 if MLP_FP8 else ml_dtypes.bfloat16
NP_SC = ml_dtypes.float8_e4m3 if SC_FP8 else ml_dtypes.bfloat16
NP_AT = ml_dtypes.float8_e4m3 if AT_FP8 else ml_dtypes.bfloat16


def _prep_w(w):
    """[d_out, d_in] f32 -> [p, j, k, c] of (WS*w.T) (k,p index d_in; j,c d_out)."""
    wt = np.asarray(w, dtype=np.float32).T
    if MLP_FP8:
        wt = wt * np.float32(WS)
    wt = wt.reshape(DT, P, DT, P).transpose(1, 2, 0, 3)
    return np.ascontiguousarray(wt).astype(NP_MLP)


def make_in_maps(sequence, Kw1, Kw2, Qw1, Qw2):
    seq = np.ascontiguousarray(np.transpose(np.asarray(sequence), (1, 0, 2)))  # [B, S, D]
    ws = {"wk1": _prep_w(Kw1), "wk2": _prep_w(Kw2),
          "wq1": _prep_w(Qw1), "wq2": _prep_w(Qw2)}
    in_maps = []
    colsums = []
    for b in range(B):
        xb = seq[b]
        xt = np.ascontiguousarray(xb.T).astype(NP_MLP)
        # [P, 2, KH, NF]: xh[p, h, q, s] = xt[(h*KH + q)*P + p, s] for s < NF
        xh = np.ascontiguousarray(
            xt[:, 0:NF].reshape(2, DT // 2, P, NF).transpose(2, 0, 1, 3))
        m = {"xn": xb.astype(NP_AT), "xt": xt, "xh": xh}
        m.update(ws)
        in_maps.append(m)
        colsums.append(xb.astype(np.float32).sum(axis=0))  # [D]
    return in_maps, colsums


def kernel(sequence, Kw1, Kw2, Qw1, Qw2):
    nc = _get_nc()
    in_maps, colsums = make_in_maps(sequence, Kw1, Kw2, Qw1, Qw2)
    res = run_bass_kernel_spmd(nc, in_maps, core_ids=list(range(B)))
    outs = []
    for b in range(B):
        ob = np.asarray(res.results[b]["out"], dtype=np.float32)
        if AT_FP8:
            # add back the rank-1 mean term removed by the W centering
            rcpv = np.asarray(res.results[b]["rcpv"], dtype=np.float32)[:, 0]
            ob = ob + np.float32(WC) * np.outer(rcpv, colsums[b])
        outs.append(ob)
    return np.stack(outs, axis=1).astype(np.float32)
